# revision 25
# baseline (speedup 1.0000x reference)
"""Multi-head causal attention with RoPE on 8 Trainium2 NeuronCores.

Sharding: 2 (batch) x 4 (head-groups of 4 heads). Each core computes
QKV projections, RoPE, flash-style causal attention and its slice of the
output projection for one batch and 4 heads; partial outputs are summed
on the host (row-sharded out_proj => partial-sum reduction).

Device layout choices (everything host-prepped to avoid on-device
transposes, fp32 has no DMA-transpose path):
  - all inputs host-packed so every DMA writes wide contiguous
    per-partition lines (>=4KB): xT in column-group-major chunks,
    W_q/W_k in per-head chunks, wv/wo as single transfers
  - Q^T, K^T computed as [head_dim, S] (lhsT = W tile, rhs = xT)
  - V computed natural [S, head_dim] (lhsT = xT tile, rhs = Wv)
  - scores computed transposed [k, q]; softmax sum over k (partitions)
    via a full-width all-ones stationary matmul, which lands the same sum
    on every PSUM partition so normalization needs no broadcast
  - RoPE rotate-half entirely on DVE via partition-offset muls against a
    host-preshifted signed sin table (no PE matmul)
  - causal diagonal 512-blocks computed at half width (cols [256:512])
    in scores/exp/PV; the half-pair's softmax contribution is folded
    into the DVE pair-sum tree with a partial add

Global schedule (software-pipelined so the PE never waits on the
softmax/normalization latency chains; every PE stall also costs a
p-state ramp-down, so stall EVENTS are what the schedule minimizes):
  - the ones tile is memset on-device (no DMA) so the input stream's
    first bulk transfer issues one Sync DIRECT2D slot earlier; wv is
    DMA'd in halves moved up the stream, and group 0's QK and first
    four V tiles run as TWO-VISIT PSUM accumulations (resume with
    start=False) so each matmul half starts as soon as its DMA chunk
    lands -- the phase-1 makespan is bound by wv's arrival
  - phase 1 emits qk projections for heads 0..2 plus head 3's first two
    seq-groups; head 3's last two groups are DEFERRED and pulled as PE
    filler into attention group 0, whose exp-latency stalls previously
    had no coverable work (out-proj filler only exists from group 1 on)
  - a single filler queue holds [deferred qk3 units, out-proj(g) units];
    attention score pairs pull from it (odd pairs only in each group's
    first head, back-loading supply), a reserve keeps the last units
    for forced pulls emitted just before each group's final-head
    softmax-sum matmul, where the PE would otherwise wait on the DVE
    accumulation chain
  - the LAST group's out-projection is split by heads: the h0+h1 partial
    is computed as filler during heads 2-3 of attention and DMA'd to
    `out`; the h2+h3 partial runs after the final normalization and goes
    to a second output `out2`; the host adds them (the device-side tail
    shrinks from 16x4 to 16x2 matmuls)
  - filler out-proj PSUM->SBUF casts run on DVE (GPSIMD/Pool has no
    PSUM port); the tail's alternate ACT / DVE since ACT has no exp
    work left after the final normalization, and each tail row-tile
    goes out as a single per-qt DMA to minimize serialized Sync issues
  - partial outputs row-buffered in SBUF and DMA'd out as bf16 rows
    (host accumulates in fp32)
"""

import math
import sys

import numpy as np

try:
    import concourse.bass as bass  # noqa: F401
except Exception:
    sys.path.insert(0, "/opt/trn_rl_repo")

import ml_dtypes

P = 128
B = 2
S = 2048
D = 2048
H = 16
HEAD = 128
N_CORES = 8
HG = 4            # head groups (tensor-parallel dimension)
HPG = H // HG     # heads per group = 4
DG = HPG * HEAD   # group width = 512
SG = 512          # q-group (free dim) size
DOUT = 2048
DI_T = D // P     # d_in k-tiles = 16
NSG = S // SG     # seq 512-groups = 4

BF16 = ml_dtypes.bfloat16


class FillerQueue:
    """Ordered queue of generators whose items are PE work units used to
    fill engine-latency stalls in the attention inner loop. A reserve
    keeps the last few units for forced pulls at each group's final
    normalization, where the PE would otherwise wait on the DVE chain."""

    def __init__(self, reserve=0):
        self.gens = []
        self.remaining = 0
        self.reserve = reserve

    def push(self, gen, n):
        self.gens.append(gen)
        self.remaining += n

    def _advance(self):
        while self.gens:
            try:
                next(self.gens[0])
                self.remaining = max(0, self.remaining - 1)
                return True
            except StopIteration:
                self.gens.pop(0)
        self.remaining = 0
        return False

    def pull(self, n=1):
        while n > 0 and self.remaining > self.reserve:
            if not self._advance():
                return
            n -= 1

    def pull_force(self, n=1):
        while n > 0 and self._advance():
            n -= 1

    def drain(self):
        while self._advance():
            pass


def _run(gen):
    for _ in gen:
        pass


def _emit(tc, io, cfg, sfx=""):
    """Emit the per-core program. io: dict of dram APs. cfg: sizes."""
    import concourse.mybir as mybir

    nc = tc.nc
    bf = mybir.dt.bfloat16
    f32 = mybir.dt.float32
    Exp = mybir.ActivationFunctionType.Exp

    s = cfg["S"]
    d = cfg["D"]
    dout = cfg["DOUT"]
    di_t = d // P          # d_in k-tiles
    st = s // P            # seq 128-tiles
    nsg = s // SG          # seq 512-groups
    nos = dout // SG       # out column slices
    inv_sqrt_hd = 1.0 / math.sqrt(HEAD)

    # host-packed layouts (see make_in_maps)
    xTg = io["xTg"].rearrange("p (g n) -> p g n", g=nsg)       # n = (o c)
    # wqk: per head, [half][q/k][o-within-half][m] so one DMA covers both
    # weight tensors (and h0 can stream in two halves)
    wqkh = io["wqkh"].rearrange("p (h n) -> p h n", h=HPG)

    const = tc.alloc_tile_pool(name="const" + sfx, bufs=1)
    stores = tc.alloc_tile_pool(name="stores" + sfx, bufs=1)

    cos_sb = const.tile([P, s], bf, tag="cos")
    sinsh_sb = const.tile([P, s], bf, tag="sinsh")
    mask_sb = const.tile([P, HG, SG], bf, tag="mask")
    ones_bf_sb = const.tile([P, P], bf, tag="ones_bf")
    wv_sb = const.tile([P, di_t, DG], bf, tag="wv")
    wo_sb = const.tile([P, HPG, dout], bf, tag="wo")

    # persistent activation stores
    qt_sb = stores.tile([P, HPG, s], bf, tag="qt")
    kt_sb = stores.tile([P, HPG, s], bf, tag="kt")
    v_sb = stores.tile([P, st, DG], bf, tag="v")
    ctx_sb = stores.tile([P, HPG, s], bf, tag="ctx")

    # pools that outlive phase 1 (the deferred h3 projections for seq
    # groups 2-3 are emitted inside attention group 0 as PE filler); they
    # live on the right-side stack so they can be released mid-phase-2
    # without violating the left stack's LIFO order
    p1tmp = tc.alloc_tile_pool(name="p1tmp" + sfx, bufs=4, side="right")
    wqkp = tc.alloc_tile_pool(name="wqk" + sfx, bufs=2, side="right")
    xtp23 = tc.alloc_tile_pool(name="xt23" + sfx, bufs=1, side="right")
    xtp01 = tc.alloc_tile_pool(name="xt01" + sfx, bufs=1, side="right")

    xt01 = xtp01.tile([P, 2, di_t * SG], bf, tag="xt01")
    xt23 = xtp23.tile([P, 2, di_t * SG], bf, tag="xt23")

    def xt(g):
        return xt01[:, g] if g < 2 else xt23[:, g - 2]

    # xt inner layout per column group: (o, c) with c the 512 cols
    hdi = di_t // 2
    hx = hdi * SG
    hw = 2 * hdi * P
    wqk_h0 = wqkp.tile([P, 2, 2, hdi, P], bf, tag="wqk")
    # DMA order = consumption order; each transfer has wide
    # per-partition lines so it runs at full DMA rate. h0 weights +
    # first column group stream in halves so the first (split) QK
    # accumulation starts as early as possible
    nc.vector.memset(ones_bf_sb[:], 1.0)
    nc.sync.dma_start(wqk_h0[:, 0], wqkh[:, 0, 0:hw])
    nc.sync.dma_start(xt01[:, 0, 0:hx], xTg[:, 0, 0:hx])
    nc.sync.dma_start(wv_sb[:, 0:hdi], io["wv"][:, 0:hdi * DG])
    nc.sync.dma_start(xt01[:, 0, hx:], xTg[:, 0, hx:])
    nc.sync.dma_start(wv_sb[:, hdi:], io["wv"][:, hdi * DG:])
    nc.sync.dma_start(wqk_h0[:, 1], wqkh[:, 0, hw:])
    nc.sync.dma_start(xt01[:, 1, :], xTg[:, 1, :])
    nc.sync.dma_start(cos_sb[:], io["cosT"][:])
    nc.sync.dma_start(sinsh_sb[:], io["sinT"][:])
    nc.sync.dma_start(mask_sb[:], io["masks"][:])
    nc.sync.dma_start(xt23[:, 0, :], xTg[:, 2, :])
    nc.sync.dma_start(xt23[:, 1, :], xTg[:, 3, :])
    nc.sync.dma_start(wo_sb[:], io["wo"][:])

    # warm the PE HAM clock gate during the input-DMA head; sized to
    # finish as the first QK operands land (~5us)
    with tc.tile_pool(name="warm" + sfx, bufs=1, space="PSUM") as warmp:
        pw = warmp.tile([P, P], f32, tag="pw")
        for i in range(72):
            nc.tensor.matmul(pw, lhsT=ones_bf_sb[:], rhs=ones_bf_sb[:],
                             start=(i == 0), stop=(i == 71))

    ps_main = tc.alloc_tile_pool(name="ps_main" + sfx, bufs=3, space="PSUM")
    psv = tc.alloc_tile_pool(name="psv" + sfx, bufs=4, space="PSUM")

    def emit_rope(qa, dst, hh, sl):
        # q*cos + rot_half(q)*sin via partition-offset DVE muls;
        # sinsh is host-preshifted: sinsh[64:] = -sin[:64], sinsh[:64] = sin[64:]
        t1 = p1tmp.tile([P, SG], bf, tag="t1")
        nc.vector.tensor_mul(t1, qa, cos_sb[:, sl])
        t2 = p1tmp.tile([P, SG], bf, tag="t2")
        nc.vector.tensor_mul(t2[0:64, :], qa[64:128, :], sinsh_sb[64:128, sl])
        nc.vector.tensor_mul(t2[64:128, :], qa[0:64, :], sinsh_sb[0:64, sl])
        nc.vector.tensor_add(dst[:, hh, sl], t1, t2)

    def gen_qk_group(h, wqk_t, g):
        # generator form: yields every 2 matmuls so deferred groups can
        # be pulled as fine-grained PE filler during attention
        sl = slice(g * SG, (g + 1) * SG)
        for wsel, dst in ((0, qt_sb), (1, kt_sb)):
            qa = p1tmp.tile([P, SG], bf, tag="qa")
            pq = ps_main.tile([P, SG], f32, tag="ps")
            for o in range(di_t):
                nc.tensor.matmul(
                    pq,
                    lhsT=wqk_t[:, o // hdi, wsel, o % hdi, :],
                    rhs=xt(g)[:, o * SG:(o + 1) * SG],
                    start=(o == 0),
                    stop=(o == di_t - 1),
                )
                if o % 2 == 1:
                    yield
            nc.scalar.copy(qa, pq)
            emit_rope(qa, dst, h, sl)

    def emit_v(si):
        g, c0 = si // 4, (si % 4) * P
        pv = ps_main.tile([P, SG], f32, tag="ps")
        for o in range(di_t):
            nc.tensor.matmul(
                pv[:, :DG],
                lhsT=xt(g)[:, o * SG + c0:o * SG + c0 + P],
                rhs=wv_sb[:, o, :],
                start=(o == 0),
                stop=(o == di_t - 1),
            )
        nc.vector.tensor_copy(v_sb[:, si, :], pv[:, :DG])

    # ---- phase 1: projections + RoPE (heads 0-2 + head 3 groups 0-1) ----
    # group 0 runs as two-visit PSUM accumulations (resume with
    # start=False) so each matmul half starts as soon as its DMA chunk
    # lands: QK on the first x/w halves, V first-halves once the first
    # half of wv arrives, then the second visits
    pqs = []
    for wsel in (0, 1):
        pq = ps_main.tile([P, SG], f32, tag="ps")
        for o in range(hdi):
            nc.tensor.matmul(pq, lhsT=wqk_h0[:, 0, wsel, o, :],
                             rhs=xt01[:, 0, o * SG:(o + 1) * SG],
                             start=(o == 0), stop=False)
        pqs.append(pq)
    pvs = []
    for si in range(4):
        c0 = (si % 4) * P
        pv = psv.tile([P, DG], f32, tag="psv")
        for o in range(hdi):
            nc.tensor.matmul(pv, lhsT=xt01[:, 0, o * SG + c0:o * SG + c0 + P],
                             rhs=wv_sb[:, o, :], start=(o == 0), stop=False)
        pvs.append(pv)
    for si in range(4):
        pv = pvs[si]
        c0 = (si % 4) * P
        for o in range(hdi, di_t):
            nc.tensor.matmul(pv, lhsT=xt01[:, 0, o * SG + c0:o * SG + c0 + P],
                             rhs=wv_sb[:, o, :], start=False,
                             stop=(o == di_t - 1))
        nc.vector.tensor_copy(v_sb[:, si, :], pv)
    for wsel, dst in ((0, qt_sb), (1, kt_sb)):
        pq = pqs[wsel]
        for o in range(hdi, di_t):
            nc.tensor.matmul(pq, lhsT=wqk_h0[:, 1, wsel, o - hdi, :],
                             rhs=xt01[:, 0, o * SG:(o + 1) * SG],
                             start=False, stop=(o == di_t - 1))
        qa = p1tmp.tile([P, SG], bf, tag="qa")
        nc.scalar.copy(qa, pq)
        emit_rope(qa, dst, 0, slice(0, SG))
    psv.release()
    ps2 = tc.alloc_tile_pool(name="ps2" + sfx, bufs=2, space="PSUM")
    ps_sum = tc.alloc_tile_pool(name="ps_sum" + sfx, bufs=1, space="PSUM")
    for g in range(1, nsg):
        _run(gen_qk_group(0, wqk_h0, g))
        for si in range(4 * g, 4 * (g + 1)):
            emit_v(si)

    for h in (1, 2):
        wqk_t = wqkp.tile([P, 2, 2, hdi, P], bf, tag="wqk")
        nc.sync.dma_start(wqk_t[:], wqkh[:, h, :])
        for g in range(nsg):
            _run(gen_qk_group(h, wqk_t, g))
    wqk_t3 = wqkp.tile([P, 2, 2, hdi, P], bf, tag="wqk")
    nc.sync.dma_start(wqk_t3[:], wqkh[:, 3, :])
    for g in (0, 1):
        _run(gen_qk_group(3, wqk_t3, g))

    xtp01.release()

    # ---- phase 2+3: attention with filler-queued output projection ----
    with tc.tile_pool(name="p2tmp" + sfx, bufs=7) as p2tmp, \
         tc.tile_pool(name="p2rb" + sfx, bufs=2) as p2rb, \
         tc.tile_pool(name="outp" + sfx, bufs=3) as outp:

        def emit_head(g, h, fq, pulls, pull_odd_only=False, end_pulls=0):
            qsl = slice(g * SG, (g + 1) * SG)
            jmax = min((g + 1) * SG // P, st)
            pctx = ps_main.tile([P, SG], f32, tag="ps")
            psum_l = ps_sum.tile([P, SG], f32, tag="l")

            ats = []
            acc = None
            npv = 0          # PV pairs emitted so far

            def emit_pv(idx):
                at2, o0, o1 = ats[idx]
                for jj, off in ((0, o0), (1, o1)):
                    j = 2 * idx + jj
                    nc.tensor.matmul(
                        pctx[:, off:],
                        lhsT=v_sb[:, j, h * P:(h + 1) * P],
                        rhs=at2[:, jj, off:],
                        start=(j == 0),
                        stop=(j == jmax - 1),
                    )

            for jp in range(0, jmax, 2):
                r = jp - 4 * g
                off0 = max(0, r) * P
                off1 = max(0, r + 1) * P
                wsl0 = slice(g * SG + off0, (g + 1) * SG)
                wsl1 = slice(g * SG + off1, (g + 1) * SG)
                ps2t = ps2.tile([P, 2, SG], f32, tag="ps2")
                nc.tensor.matmul(
                    ps2t[:, 0, off0:],
                    lhsT=kt_sb[:, h, jp * P:(jp + 1) * P],
                    rhs=qt_sb[:, h, wsl0],
                    start=True,
                    stop=True,
                )
                nc.tensor.matmul(
                    ps2t[:, 1, off1:],
                    lhsT=kt_sb[:, h, (jp + 1) * P:(jp + 2) * P],
                    rhs=qt_sb[:, h, wsl1],
                    start=True,
                    stop=True,
                )
                at2 = p2tmp.tile([P, 2, SG], bf, tag="at")
                if r < 0:
                    nc.scalar.activation(at2[:, :, :], ps2t[:, :, :],
                                         Exp, scale=inv_sqrt_hd)
                    if acc is None:
                        acc = p2tmp.tile([P, SG], bf, tag="dacc", bufs=3)
                        nc.vector.tensor_add(acc, at2[:, 0, :], at2[:, 1, :])
                    else:
                        nc.vector.tensor_add(acc, acc, at2[:, 0, :])
                        nc.vector.tensor_add(acc, acc, at2[:, 1, :])
                else:
                    nc.scalar.activation(at2[:, 0, off0:], ps2t[:, 0, off0:],
                                         Exp, scale=inv_sqrt_hd)
                    nc.scalar.activation(at2[:, 1, off1:], ps2t[:, 1, off1:],
                                         Exp, scale=inv_sqrt_hd)
                    nc.vector.tensor_mul(at2[:, 0, off0:], at2[:, 0, off0:],
                                         mask_sb[:, r, off0:])
                    nc.vector.tensor_mul(at2[:, 1, off1:], at2[:, 1, off1:],
                                         mask_sb[:, r + 1, off1:])
                    if acc is None:
                        acc = p2tmp.tile([P, SG], bf, tag="dacc", bufs=3)
                        nc.vector.tensor_copy(acc, at2[:, 0, :])
                    else:
                        nc.vector.tensor_add(acc[:, off0:], acc[:, off0:],
                                             at2[:, 0, off0:])
                    nc.vector.tensor_add(acc[:, off1:], acc[:, off1:],
                                         at2[:, 1, off1:])
                ats.append((at2, off0, off1))
                # PV trails scores by 2 pairs; filler units slot in
                # between pairs to fill ACT-paced stalls with PE work
                if len(ats) - npv > 2:
                    emit_pv(npv)
                    npv += 1
                if not pull_odd_only or len(ats) % 2 == 1:
                    fq.pull(pulls)
            while npv < len(ats):
                emit_pv(npv)
                npv += 1
            fq.pull_force(end_pulls)
            nc.tensor.matmul(psum_l, lhsT=ones_bf_sb[:], rhs=acc,
                             start=True, stop=True)
            rec = p2rb.tile([P, SG], f32, tag="rec")
            if g == nsg - 1 and h == HPG - 1:
                for c in range(4):
                    cs = slice(c * P, (c + 1) * P)
                    nc.vector.reciprocal_approx_fast(rec[:, cs], psum_l[:, cs])
                    nc.vector.tensor_mul(ctx_sb[:, h, g * SG + c * P:
                                                g * SG + (c + 1) * P],
                                         pctx[:, cs], rec[:, cs])
            else:
                nc.vector.reciprocal_approx_fast(rec, psum_l)
                nc.vector.tensor_mul(ctx_sb[:, h, qsl], pctx, rec)

        def gen_po(g, hs, dst, row0, qts=None):
            # out-proj for q-rows of group g over heads `hs`, one yield
            # per (qt, dsl) unit
            nh = len(hs)
            for qt in (qts if qts is not None else range(4 * g, 4 * (g + 1))):
                ob = outp.tile([P, dout], bf, tag="ob")
                for dsl in range(nos):
                    po = ps_main.tile([P, SG], f32, tag="ps")
                    for i, h in enumerate(hs):
                        nc.tensor.matmul(
                            po,
                            lhsT=ctx_sb[:, h, qt * P:(qt + 1) * P],
                            rhs=wo_sb[:, h, dsl * SG:(dsl + 1) * SG],
                            start=(i == 0),
                            stop=(i == nh - 1),
                        )
                    # NOTE: GPSIMD/Pool has no PSUM port; PSUM evacuation
                    # casts must stay on DVE (or ACT, which is exp-loaded
                    # during attention)
                    nc.vector.tensor_copy(ob[:, dsl * SG:(dsl + 1) * SG], po)
                    if dsl % 2 == 1:
                        c0 = (dsl - 1) * SG
                        nc.sync.dma_start(
                            dst[(qt - row0) * P:(qt - row0 + 1) * P,
                                c0:c0 + 2 * SG],
                            ob[:, c0:c0 + 2 * SG],
                        )
                    yield

        fq = FillerQueue(reserve=3)
        fq.push(gen_qk_group(3, wqk_t3, 2), 16)
        fq.push(gen_qk_group(3, wqk_t3, 3), 16)

        for g in range(nsg):
            for h in range(HPG):
                last_h = h == HPG - 1
                emit_head(g, h, fq, pulls=(4 if g == 0 else 1),
                          pull_odd_only=(g >= 1 and h == 0),
                          end_pulls=(3 if (g == nsg - 1 and last_h)
                                     else (2 if last_h else 0)))
                if g == nsg - 1 and h == 1:
                    # h0+h1 partial of the last group's out-proj runs as
                    # filler during its heads 2-3 (host adds out2)
                    fq.push(gen_po(g, (0, 1), io["out"], 0), 16)
            fq.drain()
            if g == 0:
                xtp23.release()
                wqkp.release()
                p1tmp.release()
            if g < nsg - 1:
                fq.push(gen_po(g, tuple(range(HPG)), io["out"], 0), 16)

        # tail: h2+h3 partial of the last group -> out2; casts alternate
        # ACT / DVE (ACT has no exp work left here); last row tile DMAs
        # per-dsl so the final transfer is small
        g = nsg - 1
        for qi in range(4):
            qt = 4 * g + qi
            ob = outp.tile([P, dout], bf, tag="ob")
            for dsl in range(nos):
                po = ps_main.tile([P, SG], f32, tag="ps")
                for i, h in enumerate((2, 3)):
                    nc.tensor.matmul(
                        po,
                        lhsT=ctx_sb[:, h, qt * P:(qt + 1) * P],
                        rhs=wo_sb[:, h, dsl * SG:(dsl + 1) * SG],
                        start=(i == 0),
                        stop=(i == 1),
                    )
                osl = slice(dsl * SG, (dsl + 1) * SG)
                if dsl % 2 == 0:
                    nc.scalar.copy(ob[:, osl], po)
                else:
                    nc.vector.tensor_copy(ob[:, osl], po)
                if qi == 3:
                    # the very last row tile streams out per-dsl so the
                    # final transfer+semaphore pipelines behind the casts
                    # instead of serializing after the last one
                    nc.sync.dma_start(io["out2"][qi * P:(qi + 1) * P, osl],
                                      ob[:, osl])
            if qi < 3:
                nc.sync.dma_start(io["out2"][qi * P:(qi + 1) * P, :],
                                  ob[:, :])

    for pool in (ps_sum, ps2, ps_main, stores, const):
        pool.release()


def build_program(cfg=None):
    import concourse.bacc as bacc
    import concourse.mybir as mybir
    import concourse.tile as tile

    cfg = cfg or {"S": S, "D": D, "DOUT": DOUT}
    bf = mybir.dt.bfloat16
    nc = bacc.Bacc()
    io = {
        "xTg": nc.dram_tensor("xTg", [P, NSG * DI_T * SG], bf, kind="ExternalInput"),
        "wqkh": nc.dram_tensor("wqkh", [P, HPG * 2 * DI_T * P], bf,
                               kind="ExternalInput"),
        "wv": nc.dram_tensor("wv", [P, DI_T * DG], bf, kind="ExternalInput"),
        "wo": nc.dram_tensor("wo", [P, HPG * DOUT], bf, kind="ExternalInput"),
        "cosT": nc.dram_tensor("cosT", [P, cfg["S"]], bf, kind="ExternalInput"),
        "sinT": nc.dram_tensor("sinT", [P, cfg["S"]], bf, kind="ExternalInput"),
        "masks": nc.dram_tensor("masks", [P, HG, SG], bf, kind="ExternalInput"),
        "out": nc.dram_tensor(
            "out", [cfg["S"], cfg["DOUT"]], bf, kind="ExternalOutput"
        ),
        # h2+h3 partial of the last seq group's rows (host adds to out)
        "out2": nc.dram_tensor(
            "out2", [SG, cfg["DOUT"]], bf, kind="ExternalOutput"
        ),
    }
    with tile.TileContext(nc) as tc:
        for rep in range(cfg.get("repeat", 1)):
            _emit(tc, io, cfg, sfx=f"_r{rep}")
    nc.finalize()
    return nc


def host_constants(s=S):
    inv = 1.0 / (10000.0 ** (np.arange(0, HEAD, 2, dtype=np.float32) / HEAD))
    pos = np.arange(s, dtype=np.float32)
    ang = pos[:, None] * inv[None, :]
    ang = np.concatenate([ang, ang], axis=-1)          # (s, HEAD)
    cosT = np.cos(ang).T.astype(BF16).copy()           # (HEAD, s)
    sinT = np.sin(ang).T.astype(np.float32)
    # preshifted signed sin for DVE rotate-half:
    #   rope[d<64]  = q[d]*cos[d] - q[d+64]*sin[d]  -> sinsh[64:] = -sin[:64]
    #   rope[d>=64] = q[d]*cos[d] + q[d-64]*sin[d]  -> sinsh[:64] =  sin[64:]
    sinsh = np.empty_like(sinT)
    sinsh[0:64] = sinT[64:128]
    sinsh[64:128] = -sinT[0:64]
    sinshT = sinsh.astype(BF16).copy()
    kk = np.arange(P)[:, None, None]
    rr = np.arange(HG)[None, :, None]
    qq = np.arange(SG)[None, None, :]
    masks = (kk <= qq - P * rr).astype(BF16)           # (P, HG, SG)
    return cosT, sinshT, masks


def make_in_maps(x, W_query, W_key, W_value, W_out):
    """Host-pack all inputs into DMA-friendly per-partition-contiguous
    layouts and build the 8 per-core input dicts."""
    cosT, sinshT, masks = host_constants()

    def pack_x(xb):
        # xTg[p, g, o, c] = x[g*SG+c, o*P+p]
        t = np.asarray(xb).reshape(NSG, SG, DI_T, P).transpose(3, 0, 2, 1)
        return np.ascontiguousarray(t).reshape(P, -1).astype(BF16)

    def pack_wqk(wq, wk, gsl):
        # wqkh[p, h, half, t, o, m] = w_t[(half*8+o)*P+p, gsl.start + h*P+m]
        hdi = DI_T // 2
        parts = []
        for w in (wq, wk):
            wg = np.asarray(w)[:, gsl]                   # [D, DG]
            t = wg.reshape(2, hdi, P, HPG, P).transpose(2, 3, 0, 1, 4)
            parts.append(t)                              # [p, h, half, o, m]
        t = np.stack(parts, axis=3)                      # [p, h, half, t, o, m]
        return np.ascontiguousarray(t).reshape(P, -1).astype(BF16)

    def pack_wv(w, gsl):
        # wv[p, o, n] = w[o*P+p, gsl.start+n]
        wg = np.asarray(w)[:, gsl]
        t = wg.reshape(DI_T, P, DG).transpose(1, 0, 2)
        return np.ascontiguousarray(t).reshape(P, -1).astype(BF16)

    def pack_wo(w, gsl):
        # wo[p, h, n] = w[gsl.start + h*P+p, n]  (row-shard of W_out)
        wg = np.asarray(w)[gsl, :]                       # [DG, DOUT]
        t = wg.reshape(HPG, P, DOUT).transpose(1, 0, 2)
        return np.ascontiguousarray(t).reshape(P, -1).astype(BF16)

    xg = [pack_x(np.asarray(x)[b]) for b in range(B)]
    in_maps = []
    for core in range(N_CORES):
        b, g = divmod(core, HG)
        gsl = slice(g * DG, (g + 1) * DG)
        in_maps.append({
            "xTg": xg[b],
            "wqkh": pack_wqk(W_query, W_key, gsl),
            "wv": pack_wv(W_value, gsl),
            "wo": pack_wo(W_out, gsl),
            "cosT": cosT, "sinT": sinshT, "masks": masks,
        })
    return in_maps


def gather_out(results):
    """Accumulate per-core partial outputs (row-sharded out_proj) plus the
    last seq group's h2+h3 partial (out2) into the full fp32 output."""
    out = np.zeros((B, S, DOUT), np.float32)
    for core in range(N_CORES):
        b = core // HG
        out[b] += results[core]["out"]
        out[b, S - SG:] += results[core]["out2"]
    return out


def kernel(x, W_query, W_key, W_value, W_out):
    from concourse.bass_utils import run_bass_kernel_spmd

    x = np.asarray(x)
    in_dtype = x.dtype
    nc = build_program()
    in_maps = make_in_maps(x, W_query, W_key, W_value, W_out)
    res = run_bass_kernel_spmd(nc, in_maps, core_ids=list(range(N_CORES)))
    out = gather_out(res.results)
    return out.astype(in_dtype, copy=False)
